# revision 22
# baseline (speedup 1.0000x reference)
"""DipNetEncoder Trainium2 kernel v4: 8-way batch-parallel, bf16.

Structure per core (B_loc=256), per half, per block:
- PHASE F, per oh-half: per-node feature matmul (weights streamed, PSUM
  [128,4,256]); y1 [128(o),256(b),128(n-pad)] evacuated ACT/DVE alternating;
  per 32-b chunk: xbar transpose y1->y2 (sync HWDGE), A-mix (A^T stationary),
  ACT copy evac (+sum accum), DVE square (+sumsq accum, output discarded into
  dead PSUM), z spilled n-layout (unpadded rows, parity-double-buffered DRAM)
  via SWDGE.  Transposes only ever read engine-written tiles and are only
  read by engines (DMA-adjacent transpose deps are not tracked by Tile).
- stats: AllReduce [96,2] sums -> s,t.
- PHASE M, per 64-b chunk: plain SWDGE reload of z, fused affine s*z+t
  (ACT Identity bias/scale | DVE tensor_scalar alternating), xbar transpose
  back (scalar HWDGE), fused relu+residual in one DVE op:
  x1 = max(u,0) + x1.
Bias terms are zeros per the spec and are ignored.
"""
import os
import numpy as np
import ml_dtypes

B = 2048
N = 81
BO_IN, PO_IN = 35, 40
EMB = 256
NUM_BLOCKS = int(os.environ.get("KERNEL_NUM_BLOCKS", "8"))
BN_EPS = 1e-5
N_CORES = 8
BL = B // N_CORES   # 256
NG = 82             # gram dim (81 nodes + ones row)
BQ = 64             # b-chunk for mix phase
NELT = float(B * EMB)

_cache = {}


def _build():
    import concourse.bacc as bacc
    import concourse.mybir as mybir
    import concourse.tile as tile

    BF = mybir.dt.bfloat16
    F32 = mybir.dt.float32
    AF = mybir.ActivationFunctionType
    ALU = mybir.AluOpType

    nc = bacc.Bacc("TRN2", target_bir_lowering=False, debug=False, num_devices=N_CORES)

    d_in = {}
    for half, cin in (("bo", BO_IN), ("po", PO_IN)):
        d_in[f"x_{half}"] = nc.dram_tensor(f"x_{half}", [BL // 64, cin, N, 64], BF, kind="ExternalInput").ap()
        d_in[f"w0_{half}"] = nc.dram_tensor(f"w0_{half}", [cin, 2, N, 128], BF, kind="ExternalInput").ap()
        if NUM_BLOCKS > 1:
            d_in[f"w_{half}"] = nc.dram_tensor(
                f"w_{half}", [NUM_BLOCKS - 1, 2, 128, 2, N, 128], BF, kind="ExternalInput").ap()
        d_in[f"g_{half}"] = nc.dram_tensor(f"g_{half}", [96, 2 * NUM_BLOCKS], F32, kind="ExternalInput").ap()
    d_in["amat"] = nc.dram_tensor("amat", [N, N], BF, kind="ExternalInput").ap()
    d_in["amn"] = nc.dram_tensor("amn", [N, N], F32, kind="ExternalInput").ap()
    d_out = {
        h: nc.dram_tensor(f"out_{h}", [128, 2 * BL, N], BF, kind="ExternalOutput").ap()
        for h in ("bo", "po")
    }

    with tile.TileContext(nc) as tc:
        with (
            tc.tile_pool(name="persist", bufs=1) as pp,
            tc.tile_pool(name="dram", bufs=1, space="DRAM") as dp,
        ):
            a_t = pp.tile([N, N], BF)
            nc.sync.dma_start(out=a_t[:], in_=d_in["amat"])
            a_mn = pp.tile([N, N], F32)
            nc.sync.dma_start(out=a_mn[:], in_=d_in["amn"])
            y_dram = dp.tile([96, 2, 2, BL, 128], BF)  # [n, parity, oh, b, o_lo]
            cc_in = dp.tile([96, 2], F32)
            cc_out = dp.tile([96, 2], F32)

            for hi, half in enumerate(("bo", "po")):
                cin = BO_IN if half == "bo" else PO_IN
                with tc.tile_pool(name=f"xh_{half}", bufs=1) as xp:
                    x1 = xp.tile([128, 2 * BL, N], BF)
                    x1v = x1[:].rearrange("p (b h) n -> p b h n", h=2)
                    gb = xp.tile([96, 2 * NUM_BLOCKS], F32)
                    nc.sync.dma_start(out=gb[:], in_=d_in[f"g_{half}"])

                    for blk in range(NUM_BLOCKS):
                        par = (hi * NUM_BLOCKS + blk) % 2
                        with tc.tile_pool(name=f"st_{half}_{blk}", bufs=1) as sp:
                            # ---------- PHASE F: featmm + gram + spill ----------
                            with (
                                tc.tile_pool(name=f"y_{half}_{blk}", bufs=1) as yp,
                                tc.tile_pool(name=f"q_{half}_{blk}", bufs=2) as qp,
                                tc.tile_pool(name=f"x0_{half}_{blk}", bufs=1) as xqp,
                                tc.tile_pool(name=f"w_{half}_{blk}", bufs=2) as wp,
                                tc.tile_pool(name=f"psA_{half}_{blk}", bufs=2, space="PSUM") as psA,
                                tc.tile_pool(name=f"psB_{half}_{blk}", bufs=2, space="PSUM") as psB,
                            ):
                                s1_parts = sp.tile([96, 64], F32)
                                s2_parts = sp.tile([96, 64], F32)
                                zi = 0
                                for oh in range(2):
                                    y1 = yp.tile([128, BL, 128], BF, tag="y1")
                                    if blk == 0:
                                        nq = BL // 64
                                        for sq in range(nq):
                                            x0s = xqp.tile([cin, N, 64], BF, tag="x0")
                                            nc.gpsimd.dma_start(
                                                out=x0s[:], in_=d_in[f"x_{half}"][sq])
                                            for g0 in range(0, N, 4):
                                                gn = min(4, N - g0)
                                                wg0 = wp.tile([cin, 4, 128], BF, tag="w0")
                                                nc.gpsimd.dma_start(
                                                    out=wg0[:, :gn, :],
                                                    in_=d_in[f"w0_{half}"][:, oh, g0:g0 + gn, :])
                                                ps = psA.tile([128, 4, 64], F32, tag="psA0")
                                                for j in range(gn):
                                                    nc.tensor.matmul(
                                                        ps[:, j], wg0[:, j, :],
                                                        x0s[:, g0 + j, :], start=True, stop=True)
                                                dst = y1[:, sq * 64:(sq + 1) * 64, g0:g0 + gn]
                                                src = ps[:, :gn].rearrange("p n b -> p b n")
                                                if (g0 // 4) % 2 == 0:
                                                    nc.scalar.activation(dst, src, AF.Copy)
                                                else:
                                                    nc.vector.tensor_copy(dst, src)
                                    else:
                                        for g0 in range(0, N, 4):
                                            gn = min(4, N - g0)
                                            wg = wp.tile([128, 2, 4, 128], BF, tag="w")
                                            nc.gpsimd.dma_start(
                                                out=wg[:, :, :gn, :],
                                                in_=d_in[f"w_{half}"][blk - 1][oh][:, :, g0:g0 + gn, :])
                                            ps = psA.tile([128, 4, BL], F32, tag="psA")
                                            for j in range(gn):
                                                for ih in range(2):
                                                    nc.tensor.matmul(
                                                        ps[:, j],
                                                        wg[:, ih, j, :],
                                                        x1v[:, :, ih, g0 + j],
                                                        start=(ih == 0), stop=(ih == 1))
                                            dst = y1[:, :, g0:g0 + gn]
                                            src = ps[:, :gn].rearrange("p n b -> p b n")
                                            if (g0 // 4) % 2 == 0:
                                                nc.scalar.activation(dst, src, AF.Copy)
                                            else:
                                                nc.vector.tensor_copy(dst, src)
                                    # fwd transpose (engine-written y1 -> xbar),
                                    # A-mix + stats accumulation, z spill to DRAM.
                                    for s in range(BL // 32):
                                        y2c = qp.tile([128, 32, 128], BF, tag="y2c")
                                        nc.sync.dma_start(
                                            out=y2c[:], in_=y1[:, s * 32:(s + 1) * 32, :],
                                            transpose=True)
                                        y2f = y2c[0:N].rearrange("n b o -> n (b o)")
                                        zst = qp.tile([96, 32 * 128], BF, tag="zst")
                                        for c in range(4):
                                            zp = psB.tile([N, 2, 512], F32, tag="psB")
                                            for k in range(2):
                                                nc.tensor.matmul(
                                                    zp[:, k], a_t[:],
                                                    y2f[:, c * 1024 + k * 512:c * 1024 + (k + 1) * 512],
                                                    start=True, stop=True)
                                            nc.scalar.activation(
                                                zst[0:N, c * 1024:(c + 1) * 1024],
                                                zp[:].rearrange("n k o -> n (k o)"), AF.Copy,
                                                accum_out=s1_parts[0:N, zi:zi + 1])
                                            zi += 1
                                        for k2 in range(4):
                                            sqp = psB.tile([N, 2, 512], F32, tag="psB")
                                            sc = 4 * (s + 8 * oh) + k2
                                            nc.vector.scalar_tensor_tensor(
                                                sqp[:].rearrange("n k o -> n (k o)"),
                                                zst[0:N, k2 * 1024:(k2 + 1) * 1024], 1.0,
                                                zst[0:N, k2 * 1024:(k2 + 1) * 1024],
                                                ALU.mult, ALU.mult,
                                                accum_out=s2_parts[0:N, sc:sc + 1])
                                        nc.gpsimd.dma_start(
                                            out=y_dram[:, par, oh, s * 32:(s + 1) * 32, :],
                                            in_=zst[:].rearrange("n (b o) -> n b o", o=128))

                                # ---------- stats: AR(sum, sumsq) -> s,t ----------
                                st = sp.tile([96, 2], F32)
                                nc.vector.memset(st[:], 0.0)
                                nc.vector.tensor_reduce(st[0:N, 0:1], s1_parts[0:N], mybir.AxisListType.X, ALU.add)
                                nc.vector.tensor_reduce(st[0:N, 1:2], s2_parts[0:N], mybir.AxisListType.X, ALU.add)
                                nc.gpsimd.dma_start(cc_in[:], st[:])
                                nc.gpsimd.collective_compute(
                                    "AllReduce", ALU.add,
                                    replica_groups=[list(range(N_CORES))],
                                    ins=[cc_in.opt()], outs=[cc_out.opt()])
                                glob = sp.tile([96, 2], F32)
                                nc.gpsimd.dma_start(glob[:], cc_out[:])
                                mz = sp.tile([N, 1], F32)
                                nc.vector.tensor_scalar_mul(mz[:], glob[0:N, 0:1], 1.0 / NELT)
                                msq = sp.tile([N, 1], F32)
                                nc.vector.scalar_tensor_tensor(msq[:], mz[:], 0.0, mz[:], ALU.bypass, ALU.mult)
                                va = sp.tile([N, 1], F32)
                                nc.vector.scalar_tensor_tensor(va[:], glob[0:N, 1:2], 1.0 / NELT, msq[:], ALU.mult, ALU.subtract)
                                nc.vector.tensor_scalar_add(va[:], va[:], BN_EPS)
                                rt = sp.tile([N, 1], F32)
                                nc.scalar.activation(rt[:], va[:], AF.Sqrt)
                                s_f = sp.tile([N, 1], F32)
                                nc.vector.reciprocal(s_f[:], rt[:])
                                nc.vector.scalar_tensor_tensor(
                                    s_f[:], s_f[:], 0.0, gb[0:N, 2 * blk:2 * blk + 1], ALU.bypass, ALU.mult)
                                t_f = sp.tile([N, 1], F32)
                                nc.vector.scalar_tensor_tensor(
                                    t_f[:], mz[:], -1.0, s_f[:], ALU.mult, ALU.mult)
                                nc.vector.scalar_tensor_tensor(
                                    t_f[:], t_f[:], 0.0, gb[0:N, 2 * blk + 1:2 * blk + 2], ALU.bypass, ALU.add)
                            # ---------- PHASE M: mix + affine + residual ----------
                            with (
                                tc.tile_pool(name=f"c_{half}_{blk}", bufs=2) as cp,
                                tc.tile_pool(name=f"z_{half}_{blk}", bufs=2) as zp_,
                                tc.tile_pool(name=f"u_{half}_{blk}", bufs=2) as up_,
                            ):
                                for oh in range(2):
                                    for q in range(BL // BQ):
                                        uc = cp.tile([96, BQ * 128], BF, tag="uc")
                                        nc.gpsimd.dma_start(
                                            out=uc[:].rearrange("n (b o) -> n b o", o=128),
                                            in_=y_dram[:, par, oh, q * BQ:(q + 1) * BQ, :])
                                        uu = zp_.tile([96, BQ * 128], BF, tag="uu")
                                        if q % 2 == 0:
                                            nc.scalar.activation(
                                                uu[0:N], uc[0:N], AF.Identity,
                                                bias=t_f[:], scale=s_f[:])
                                        else:
                                            nc.vector.tensor_scalar(
                                                uu[0:N], uc[0:N], s_f[:], t_f[:], ALU.mult, ALU.add)
                                        ul = up_.tile([128, BQ, 96], BF, tag="ul")
                                        nc.scalar.dma_start(out=ul[:], in_=uu[:], transpose=True)
                                        xs = x1[:, 2 * q * BQ + oh: 2 * (q + 1) * BQ + oh - 1: 2, :]
                                        if blk == 0:
                                            nc.vector.tensor_scalar_max(xs, ul[:, :, 0:N], 0.0)
                                        else:
                                            nc.vector.scalar_tensor_tensor(
                                                xs, ul[:, :, 0:N], 0.0, xs, ALU.max, ALU.add)
                    nc.sync.dma_start(out=d_out[half], in_=x1[:])
    nc.finalize()
    return nc


def _prep_inputs(inputs):
    bf = ml_dtypes.bfloat16
    A = np.asarray(inputs["A"], np.float32)
    amat = np.ascontiguousarray(A.T).astype(bf)
    amn = np.ascontiguousarray(A).astype(np.float32)
    halves = {}
    for half in ("bo", "po"):
        pre = "board" if half == "bo" else "po"
        cin = BO_IN if half == "bo" else PO_IN
        w0 = np.asarray(inputs[f"{pre}_W0"], np.float32)
        # [n, i, o] -> [i, oh, n, o_lo]
        w0p = w0.reshape(N, cin, 2, 128).transpose(1, 2, 0, 3)
        d = {f"w0_{half}": np.ascontiguousarray(w0p).astype(bf)}
        if NUM_BLOCKS > 1:
            w = np.asarray(inputs[f"{pre}_W"], np.float32)[:NUM_BLOCKS - 1]
            # [blk, n, i, o] -> [blk, oh, i_lo, ih, n, o_lo]
            wp = w.reshape(NUM_BLOCKS - 1, N, 2, 128, 2, 128).transpose(0, 4, 3, 2, 1, 5)
            d[f"w_{half}"] = np.ascontiguousarray(wp).astype(bf)
        gam = np.asarray(inputs[f"{pre}_gamma"], np.float32)[:NUM_BLOCKS]
        bet = np.asarray(inputs[f"{pre}_beta"], np.float32)[:NUM_BLOCKS]
        g = np.zeros((96, 2 * NUM_BLOCKS), np.float32)
        g[:N, 0::2] = gam.T
        g[:N, 1::2] = bet.T
        d[f"g_{half}"] = g
        halves[half] = d
    core_ins = []
    for c in range(N_CORES):
        m = {"amat": amat, "amn": amn}
        for half in ("bo", "po"):
            m.update(halves[half])
            x = np.asarray(inputs["x_bo" if half == "bo" else "x_po"], np.float32)
            xs = x[c * BL:(c + 1) * BL]
            xt = xs.transpose(2, 1, 0).reshape(-1, N, 4, 64).transpose(2, 0, 1, 3)
            m[f"x_{half}"] = np.ascontiguousarray(xt).astype(bf)
        core_ins.append(m)
    return core_ins


def _unpack_outputs(results):
    out = np.zeros((B, N, 2 * EMB), np.float32)
    for c in range(N_CORES):
        for j, half in enumerate(("bo", "po")):
            r = np.asarray(results[c][f"out_{half}"]).astype(np.float32)
            r = r.reshape(128, BL, 2, N)
            out[c * BL:(c + 1) * BL, :, j * EMB:(j + 1) * EMB] = \
                r.transpose(1, 3, 2, 0).reshape(BL, N, EMB)
    return out


def kernel(**inputs) -> np.ndarray:
    from concourse.bass_utils import run_bass_kernel_spmd

    if "nc" not in _cache:
        _cache["nc"] = _build()
    core_ins = _prep_inputs(inputs)
    res = run_bass_kernel_spmd(_cache["nc"], core_ins, core_ids=list(range(N_CORES)),
                               trace=bool(int(os.environ.get("KERNEL_TRACE", "0"))))
    _cache["last_result"] = res
    return _unpack_outputs(res.results)
